# revision 13
# baseline (speedup 1.0000x reference)
"""Trainium2 Bass kernel for the DGCNN-style EdgeConv layer (KNN graph +
1x1 conv + BN + LeakyReLU + max over neighbors).

Math: for each batch b and point n,
  out[b, :, n] = lrelu( max_{m in KNN16(n)} u[m, :] + v[n, :] )
where u[m, :] = inv * (W1 @ x_m),  v[n, :] = inv * ((W2 - W1) @ x_n) + c,
W = [W1 W2] the 1x1-conv weight, inv/c the folded BN affine. LeakyReLU
commutes with the max since it is monotone, and the conv of
[nbr - ctr, ctr] splits into the u/v terms above, so only the KNN
selection and a 16-row gather+max remain data-dependent.

Device pipeline per core (one batch-half, 4096 query rows):
  - PE computes score chunks s[n, m] = 2<x_n, x_m> - |x_m|^2 via an
    augmented-contraction matmul (65th row carries -|x_m|^2).
  - ACT evicts PSUM -> SBUF.
  - DVE per 512-chunk: max8 (top-8 values) + max_index (their in-chunk
    positions) -> 128 candidates/row; candidates are packed as
    round(4*s) + (8191 - global_pos) * 2^-13 (exact in fp32) so one
    max8 chain on the packed array yields the top-17 values AND their
    global indices with jax-compatible tie-breaking. Slot 0 is provably
    the diagonal (self-match), which is dropped -> 16 neighbor indices.
  - Neighbor u-rows are fetched with a gpsimd dma_gather (512B rows)
    and max-reduced pairwise; v is added, LeakyReLU applied, and the
    [n, o] block DMA'd out (final transpose to [o, n] happens on host).
"""

import numpy as np

B, C, N, O, K = 4, 64, 8192, 128, 16
NCORES = 8
HALF = N // 2        # query rows per core
NBLK = HALF // 128   # 32 row blocks
CHUNK = 512
NCHUNK = N // CHUNK  # 16
BN_EPS = 1e-5
LRELU = 0.2
CBIG = 12582912.0    # 1.5 * 2^23: fp32 add forces round-to-integer
NEG = -3.0e38

_CACHED = {}


def _build_bass(finalize=True, stage='full', precision='f32r'):
    import concourse.bacc as bacc
    import concourse.tile as tile
    from concourse import mybir

    f32 = mybir.dt.float32
    i16 = mybir.dt.int16
    u16 = mybir.dt.uint16
    f32r = mybir.dt.float32r
    Alu = mybir.AluOpType

    AUG = 65 if precision == 'fp32' else 66
    mmdt = f32 if precision == 'fp32' else f32r
    nc = bacc.Bacc()
    lhs_d = nc.declare_dram_parameter("lhs_aug", [AUG, HALF], mmdt, isOutput=False)
    rhs_d = nc.declare_dram_parameter("rhs_aug", [AUG, N], mmdt, isOutput=False)
    u_d = nc.declare_dram_parameter("u", [N, O], f32, isOutput=False)
    v_d = nc.declare_dram_parameter("v", [128, HALF], f32, isOutput=False)
    cb_d = nc.declare_dram_parameter("cbase", [128, 128], f32, isOutput=False)
    y_d = nc.declare_dram_parameter("y", [HALF, O], f32, isOutput=True)
    bounce_d = nc.dram_tensor("bounce", [NBLK, 16, 128], i16)

    with tile.TileContext(nc) as tc:
        with tc.tile_pool(name="const", bufs=1) as constp, \
             tc.tile_pool(name="score", bufs=2) as scorep, \
             tc.tile_pool(name="psum", bufs=8, space="PSUM") as psump, \
             tc.tile_pool(name="cand", bufs=2) as candp, \
             tc.tile_pool(name="small", bufs=2) as smallp, \
             tc.tile_pool(name="gather", bufs=2) as gatherp:

            lhs = constp.tile([AUG, HALF], mmdt)
            nc.sync.dma_start(lhs[:], lhs_d[:])
            rhs = constp.tile([AUG, N], mmdt)
            nc.sync.dma_start(rhs[:], rhs_d[:])
            v_sb = constp.tile([128, HALF], f32)
            nc.sync.dma_start(v_sb[:], v_d[:])
            cbase = constp.tile([128, 128], f32)
            nc.sync.dma_start(cbase[:], cb_d[:])

            for blk in range(NBLK):
                s_sb = scorep.tile([128, N], f32, tag="s")
                cand = candp.tile([128, 128], f32, tag="cv")
                cpos = candp.tile([128, 128], u16, tag="cp")
                lhsT = lhs[:, blk * 128:(blk + 1) * 128]
                for c in range(NCHUNK):
                    ps = psump.tile([128, CHUNK], f32, tag="ps")
                    nc.tensor.matmul(ps[:], lhsT,
                                     rhs[:, c * CHUNK:(c + 1) * CHUNK],
                                     start=True, stop=True)
                    sc = s_sb[:, c * CHUNK:(c + 1) * CHUNK]
                    nc.scalar.copy(sc, ps[:])
                    nc.vector.max(cand[:, c * 8:(c + 1) * 8], sc)
                    nc.vector.max_index(cpos[:, c * 8:(c + 1) * 8],
                                        cand[:, c * 8:(c + 1) * 8], sc)

                # gcand[slot] = global index of candidate slot
                cp_f = smallp.tile([128, 128], f32, tag="cpf")
                nc.vector.tensor_copy(cp_f[:], cpos[:])
                gcand = smallp.tile([128, 128], f32, tag="gcand")
                nc.vector.tensor_add(gcand[:], cp_f[:], cbase[:])

                # top-17 chain on a copy of the exact candidate values
                candc = smallp.tile([128, 128], f32, tag="candc")
                nc.vector.tensor_copy(candc[:], cand[:])
                w = smallp.tile([128, 24], f32, tag="w")
                nc.vector.max(w[:, 0:8], candc[:])
                nc.vector.match_replace(candc[:], w[:, 0:8], candc[:], NEG)
                nc.vector.max(w[:, 8:16], candc[:])
                nc.vector.match_replace(candc[:], w[:, 8:16], candc[:], NEG)
                nc.vector.max(w[:, 16:24], candc[:])

                # extract winners' global indices: for rank j (1..16, rank 0 is
                # the diagonal self-match), gm_j = sum(gcand * (cand == w_j))
                gm = smallp.tile([128, K], f32, tag="gm")
                dummy = smallp.tile([128, 128], f32, tag="dummy")
                for j in range(1, 17):
                    nc.vector.scalar_tensor_tensor(
                        dummy[:], cand[:], w[:, j:j + 1], gcand[:],
                        Alu.is_equal, Alu.mult,
                        accum_out=gm[:, j - 1:j])

                # neighbor indices -> int16, bounce via DRAM to the k-major
                # 16-partition-wrapped layout dma_gather expects
                mi = smallp.tile([128, K], i16, tag="mi")
                nc.vector.tensor_copy(mi[:], gm[:])
                # bounce out writes the 16-partition-wrapped layout directly:
                # wrapped[q, 8k + nh] = m[16 nh + q, k]
                nc.sync.dma_start(
                    bounce_d[blk].rearrange("q (k nh) -> nh q k", nh=8), mi[:])
                idxsb = smallp.tile([128, 128], i16, tag="idx")
                if stage != 'nobounce':
                    bview8 = bounce_d[blk].unsqueeze(0).to_broadcast([8, 16, 128])
                    nc.sync.dma_start(idxsb[:], bview8)
                else:
                    nc.vector.memset(idxsb[:], 0)

                # dma_gather is limited to 1024 indices per call -> 2 calls
                ug = gatherp.tile([128, K * O], f32, tag="ug")
                if stage in ('full', 'nobounce'):
                    for half in range(2):
                        nc.gpsimd.dma_gather(
                            ug[:, half * 8 * O:(half + 1) * 8 * O].rearrange(
                                "p (k o) -> p k o", o=O),
                            u_d[:],
                            idxsb[:, half * 64:(half + 1) * 64],
                            num_idxs=8 * 128,
                            num_idxs_reg=8 * 128,
                            elem_size=O,
                            queue_num=0,
                        )
                else:
                    nc.vector.memset(ug[:], 0.0)

                # max over the 16 gathered u-rows (pairwise tree)
                r1 = gatherp.tile([128, 8 * O], f32, tag="r1")
                nc.vector.tensor_max(r1[:], ug[:, 0:8 * O], ug[:, 8 * O:16 * O])
                r2 = gatherp.tile([128, 4 * O], f32, tag="r2")
                nc.vector.tensor_max(r2[:], r1[:, 0:4 * O], r1[:, 4 * O:8 * O])
                r3 = gatherp.tile([128, 2 * O], f32, tag="r3")
                nc.vector.tensor_max(r3[:], r2[:, 0:2 * O], r2[:, 2 * O:4 * O])
                s16 = gatherp.tile([128, O], f32, tag="s16")
                nc.vector.tensor_max(s16[:], r3[:, 0:O], r3[:, O:2 * O])

                # y = lrelu(s16 + v)
                y1 = gatherp.tile([128, O], f32, tag="y1")
                nc.vector.tensor_add(y1[:], s16[:], v_sb[:, blk * 128:(blk + 1) * 128])
                yb = gatherp.tile([128, O], f32, tag="yb")
                nc.vector.scalar_tensor_tensor(yb[:], y1[:], LRELU, y1[:],
                                               Alu.mult, Alu.max)
                nc.sync.dma_start(y_d[blk * 128:(blk + 1) * 128, :], yb[:])

    if finalize:
        nc.finalize()
    return nc


def _rne11(a):
    b = np.ascontiguousarray(a, np.float32).view(np.uint32).copy()
    lsb = (b >> 12) & 1
    b = ((b + 2047 + lsb) >> 12) << 12
    return b.view(np.float32)


def _host_prep(x, conv_w, bn_gamma, bn_beta, bn_mean, bn_var, precision='f32r'):
    f32 = np.float32
    inv = (bn_gamma / np.sqrt(bn_var + BN_EPS)).astype(f32)
    cvec = (bn_beta - bn_mean * inv).astype(f32)
    W1 = conv_w[:, :C].astype(f32)
    W2 = conv_w[:, C:].astype(f32)
    cbase = np.broadcast_to(
        (CHUNK * (np.arange(128) // 8)).astype(f32)[None, :],
        (128, 128)).astype(f32).copy()
    in_maps = []
    for core in range(NCORES):
        b, h = core // 2, core % 2
        xb = np.asarray(x[b], dtype=f32)                       # [C, N]
        sq = (xb * xb).sum(0, dtype=f32)
        if precision == 'fp32':
            lhs_aug = np.concatenate(
                [2.0 * xb[:, h * HALF:(h + 1) * HALF], np.ones((1, HALF), f32)], 0)
            rhs_aug = np.concatenate([xb, -sq[None, :]], 0)
        else:
            # consistent rounded-cloud geometry: selection = exact KNN of the
            # f32r-rounded points; norms split hi/lo so they stay f32r-exact
            xr = _rne11(xb)
            sqr = (xr * xr).sum(0, dtype=np.float64).astype(f32)
            sq_hi = _rne11(sqr)
            sq_lo = _rne11(sqr - sq_hi)
            ones = np.ones((1, HALF), f32)
            lhs_aug = np.concatenate(
                [2.0 * xr[:, h * HALF:(h + 1) * HALF], ones, ones], 0)
            rhs_aug = np.concatenate([xr, -sq_hi[None, :], -sq_lo[None, :]], 0)
        u = (xb.T @ W1.T) * inv[None, :]                       # [N, O]
        vfull = (xb.T @ (W2 - W1).T) * inv[None, :] + cvec[None, :]
        vh = vfull[h * HALF:(h + 1) * HALF]                    # [HALF, O]
        v_sb = vh.reshape(NBLK, 128, O).transpose(1, 0, 2).reshape(128, HALF)
        in_maps.append({
            "lhs_aug": np.ascontiguousarray(lhs_aug, dtype=f32),
            "rhs_aug": np.ascontiguousarray(rhs_aug, dtype=f32),
            "u": np.ascontiguousarray(u, dtype=f32),
            "v": np.ascontiguousarray(v_sb, dtype=f32),
            "cbase": cbase,
        })
    return in_maps


PRECISION = 'f32r'


def kernel(x, conv_w, bn_gamma, bn_beta, bn_mean, bn_var):
    from concourse.bass_utils import run_bass_kernel_spmd

    x = np.asarray(x)
    in_maps = _host_prep(np.asarray(x, np.float32), np.asarray(conv_w),
                         np.asarray(bn_gamma), np.asarray(bn_beta),
                         np.asarray(bn_mean), np.asarray(bn_var),
                         precision=PRECISION)
    if "nc" not in _CACHED:
        _CACHED["nc"] = _build_bass(precision=PRECISION)
    res = run_bass_kernel_spmd(_CACHED["nc"], in_maps, list(range(NCORES)))
    out = np.empty((B, O, N), np.float32)
    for core in range(NCORES):
        b, h = core // 2, core % 2
        out[b, :, h * HALF:(h + 1) * HALF] = res.results[core]["y"].T
    return out


# revision 14
# speedup vs baseline: 1.0065x; 1.0065x over previous
"""Trainium2 Bass kernel for the DGCNN-style EdgeConv layer (KNN graph +
1x1 conv + BN + LeakyReLU + max over neighbors).

Math: for each batch b and point n,
  out[b, :, n] = lrelu( max_{m in KNN16(n)} u[m, :] + v[n, :] )
where u[m, :] = inv * (W1 @ x_m),  v[n, :] = inv * ((W2 - W1) @ x_n) + c,
W = [W1 W2] the 1x1-conv weight, inv/c the folded BN affine. LeakyReLU
commutes with the max since it is monotone, and the conv of
[nbr - ctr, ctr] splits into the u/v terms above, so only the KNN
selection and a 16-row gather+max remain data-dependent.

Device pipeline per core (one batch-half, 4096 query rows):
  - PE computes score chunks s[n, m] = 2<x_n, x_m> - |x_m|^2 via an
    augmented-contraction matmul (65th row carries -|x_m|^2).
  - ACT evicts PSUM -> SBUF.
  - DVE per 512-chunk: max8 (top-8 values) + max_index (their in-chunk
    positions) -> 128 candidates/row; candidates are packed as
    round(4*s) + (8191 - global_pos) * 2^-13 (exact in fp32) so one
    max8 chain on the packed array yields the top-17 values AND their
    global indices with jax-compatible tie-breaking. Slot 0 is provably
    the diagonal (self-match), which is dropped -> 16 neighbor indices.
  - Neighbor u-rows are fetched with a gpsimd dma_gather (512B rows)
    and max-reduced pairwise; v is added, LeakyReLU applied, and the
    [n, o] block DMA'd out (final transpose to [o, n] happens on host).
"""

import numpy as np

B, C, N, O, K = 4, 64, 8192, 128, 16
NCORES = 8
HALF = N // 2        # query rows per core
NBLK = HALF // 128   # 32 row blocks
CHUNK = 512
NCHUNK = N // CHUNK  # 16
BN_EPS = 1e-5
LRELU = 0.2
CBIG = 12582912.0    # 1.5 * 2^23: fp32 add forces round-to-integer
NEG = -3.0e38

_CACHED = {}


def _build_bass(finalize=True, stage='full', precision='f32r'):
    import concourse.bacc as bacc
    import concourse.tile as tile
    from concourse import mybir

    f32 = mybir.dt.float32
    i16 = mybir.dt.int16
    u16 = mybir.dt.uint16
    f32r = mybir.dt.float32r
    Alu = mybir.AluOpType

    AUG = 65 if precision == 'fp32' else 66
    mmdt = f32 if precision == 'fp32' else f32r
    nc = bacc.Bacc(num_swdge_queues=4)
    lhs_d = nc.declare_dram_parameter("lhs_aug", [AUG, HALF], mmdt, isOutput=False)
    rhs_d = nc.declare_dram_parameter("rhs_aug", [AUG, N], mmdt, isOutput=False)
    u_d = nc.declare_dram_parameter("u", [N, O], f32, isOutput=False)
    v_d = nc.declare_dram_parameter("v", [128, HALF], f32, isOutput=False)
    cb_d = nc.declare_dram_parameter("cbase", [128, 128], f32, isOutput=False)
    y_d = nc.declare_dram_parameter("y", [HALF, O], f32, isOutput=True)
    bounce_d = nc.dram_tensor("bounce", [NBLK, 16, 128], i16)

    with tile.TileContext(nc) as tc:
        with tc.tile_pool(name="const", bufs=1) as constp, \
             tc.tile_pool(name="score", bufs=2) as scorep, \
             tc.tile_pool(name="psum", bufs=8, space="PSUM") as psump, \
             tc.tile_pool(name="cand", bufs=2) as candp, \
             tc.tile_pool(name="small", bufs=2) as smallp, \
             tc.tile_pool(name="gather", bufs=2) as gatherp:

            lhs = constp.tile([AUG, HALF], mmdt)
            nc.sync.dma_start(lhs[:], lhs_d[:])
            rhs = constp.tile([AUG, N], mmdt)
            nc.sync.dma_start(rhs[:], rhs_d[:])
            v_sb = constp.tile([128, HALF], f32)
            nc.sync.dma_start(v_sb[:], v_d[:])
            cbase = constp.tile([128, 128], f32)
            nc.sync.dma_start(cbase[:], cb_d[:])

            for blk in range(NBLK):
                s_sb = scorep.tile([128, N], f32, tag="s")
                cand = candp.tile([128, 128], f32, tag="cv")
                cpos = candp.tile([128, 128], u16, tag="cp")
                lhsT = lhs[:, blk * 128:(blk + 1) * 128]
                for c in range(NCHUNK):
                    ps = psump.tile([128, CHUNK], f32, tag="ps")
                    nc.tensor.matmul(ps[:], lhsT,
                                     rhs[:, c * CHUNK:(c + 1) * CHUNK],
                                     start=True, stop=True)
                    sc = s_sb[:, c * CHUNK:(c + 1) * CHUNK]
                    nc.scalar.copy(sc, ps[:])
                    nc.vector.max(cand[:, c * 8:(c + 1) * 8], sc)
                    nc.vector.max_index(cpos[:, c * 8:(c + 1) * 8],
                                        cand[:, c * 8:(c + 1) * 8], sc)

                # gcand[slot] = global index of candidate slot
                cp_f = smallp.tile([128, 128], f32, tag="cpf")
                nc.vector.tensor_copy(cp_f[:], cpos[:])
                gcand = smallp.tile([128, 128], f32, tag="gcand")
                nc.vector.tensor_add(gcand[:], cp_f[:], cbase[:])

                # top-17 chain on a copy of the exact candidate values
                candc = smallp.tile([128, 128], f32, tag="candc")
                nc.vector.tensor_copy(candc[:], cand[:])
                w = smallp.tile([128, 24], f32, tag="w")
                nc.vector.max(w[:, 0:8], candc[:])
                nc.vector.match_replace(candc[:], w[:, 0:8], candc[:], NEG)
                nc.vector.max(w[:, 8:16], candc[:])
                nc.vector.match_replace(candc[:], w[:, 8:16], candc[:], NEG)
                nc.vector.max(w[:, 16:24], candc[:])

                # extract winners' global indices: for rank j (1..16, rank 0 is
                # the diagonal self-match), gm_j = sum(gcand * (cand == w_j))
                gm = smallp.tile([128, K], f32, tag="gm")
                dummy = smallp.tile([128, 128], f32, tag="dummy")
                for j in range(1, 17):
                    nc.vector.scalar_tensor_tensor(
                        dummy[:], cand[:], w[:, j:j + 1], gcand[:],
                        Alu.is_equal, Alu.mult,
                        accum_out=gm[:, j - 1:j])

                # neighbor indices -> int16, bounce via DRAM to the k-major
                # 16-partition-wrapped layout dma_gather expects
                mi = smallp.tile([128, K], i16, tag="mi")
                nc.vector.tensor_copy(mi[:], gm[:])
                # bounce out writes the 16-partition-wrapped layout directly:
                # wrapped[q, 8k + nh] = m[16 nh + q, k]
                nc.sync.dma_start(
                    bounce_d[blk].rearrange("q (k nh) -> nh q k", nh=8), mi[:])
                idxsb = smallp.tile([128, 128], i16, tag="idx")
                if stage != 'nobounce':
                    bview8 = bounce_d[blk].unsqueeze(0).to_broadcast([8, 16, 128])
                    nc.sync.dma_start(idxsb[:], bview8)
                else:
                    nc.vector.memset(idxsb[:], 0)

                # dma_gather is limited to 1024 indices per call -> 2 calls
                ug = gatherp.tile([128, K * O], f32, tag="ug")
                if stage in ('full', 'nobounce'):
                    for half in range(2):
                        nc.gpsimd.dma_gather(
                            ug[:, half * 8 * O:(half + 1) * 8 * O].rearrange(
                                "p (k o) -> p k o", o=O),
                            u_d[:],
                            idxsb[:, half * 64:(half + 1) * 64],
                            num_idxs=8 * 128,
                            num_idxs_reg=8 * 128,
                            elem_size=O,
                            queue_num=blk % 4,
                        )
                else:
                    nc.vector.memset(ug[:], 0.0)

                # max over the 16 gathered u-rows (pairwise tree)
                r1 = gatherp.tile([128, 8 * O], f32, tag="r1")
                nc.vector.tensor_max(r1[:], ug[:, 0:8 * O], ug[:, 8 * O:16 * O])
                r2 = gatherp.tile([128, 4 * O], f32, tag="r2")
                nc.vector.tensor_max(r2[:], r1[:, 0:4 * O], r1[:, 4 * O:8 * O])
                r3 = gatherp.tile([128, 2 * O], f32, tag="r3")
                nc.vector.tensor_max(r3[:], r2[:, 0:2 * O], r2[:, 2 * O:4 * O])
                s16 = gatherp.tile([128, O], f32, tag="s16")
                nc.vector.tensor_max(s16[:], r3[:, 0:O], r3[:, O:2 * O])

                # y = lrelu(s16 + v)
                y1 = gatherp.tile([128, O], f32, tag="y1")
                nc.vector.tensor_add(y1[:], s16[:], v_sb[:, blk * 128:(blk + 1) * 128])
                yb = gatherp.tile([128, O], f32, tag="yb")
                nc.vector.scalar_tensor_tensor(yb[:], y1[:], LRELU, y1[:],
                                               Alu.mult, Alu.max)
                nc.sync.dma_start(y_d[blk * 128:(blk + 1) * 128, :], yb[:])

    if finalize:
        nc.finalize()
    return nc


def _rne11(a):
    b = np.ascontiguousarray(a, np.float32).view(np.uint32).copy()
    lsb = (b >> 12) & 1
    b = ((b + 2047 + lsb) >> 12) << 12
    return b.view(np.float32)


def _host_prep(x, conv_w, bn_gamma, bn_beta, bn_mean, bn_var, precision='f32r'):
    f32 = np.float32
    inv = (bn_gamma / np.sqrt(bn_var + BN_EPS)).astype(f32)
    cvec = (bn_beta - bn_mean * inv).astype(f32)
    W1 = conv_w[:, :C].astype(f32)
    W2 = conv_w[:, C:].astype(f32)
    cbase = np.broadcast_to(
        (CHUNK * (np.arange(128) // 8)).astype(f32)[None, :],
        (128, 128)).astype(f32).copy()
    in_maps = []
    for core in range(NCORES):
        b, h = core // 2, core % 2
        xb = np.asarray(x[b], dtype=f32)                       # [C, N]
        sq = (xb * xb).sum(0, dtype=f32)
        if precision == 'fp32':
            lhs_aug = np.concatenate(
                [2.0 * xb[:, h * HALF:(h + 1) * HALF], np.ones((1, HALF), f32)], 0)
            rhs_aug = np.concatenate([xb, -sq[None, :]], 0)
        else:
            # consistent rounded-cloud geometry: selection = exact KNN of the
            # f32r-rounded points; norms split hi/lo so they stay f32r-exact
            xr = _rne11(xb)
            sqr = (xr * xr).sum(0, dtype=np.float64).astype(f32)
            sq_hi = _rne11(sqr)
            sq_lo = _rne11(sqr - sq_hi)
            ones = np.ones((1, HALF), f32)
            lhs_aug = np.concatenate(
                [2.0 * xr[:, h * HALF:(h + 1) * HALF], ones, ones], 0)
            rhs_aug = np.concatenate([xr, -sq_hi[None, :], -sq_lo[None, :]], 0)
        u = (xb.T @ W1.T) * inv[None, :]                       # [N, O]
        vfull = (xb.T @ (W2 - W1).T) * inv[None, :] + cvec[None, :]
        vh = vfull[h * HALF:(h + 1) * HALF]                    # [HALF, O]
        v_sb = vh.reshape(NBLK, 128, O).transpose(1, 0, 2).reshape(128, HALF)
        in_maps.append({
            "lhs_aug": np.ascontiguousarray(lhs_aug, dtype=f32),
            "rhs_aug": np.ascontiguousarray(rhs_aug, dtype=f32),
            "u": np.ascontiguousarray(u, dtype=f32),
            "v": np.ascontiguousarray(v_sb, dtype=f32),
            "cbase": cbase,
        })
    return in_maps


PRECISION = 'f32r'


def kernel(x, conv_w, bn_gamma, bn_beta, bn_mean, bn_var):
    from concourse.bass_utils import run_bass_kernel_spmd

    x = np.asarray(x)
    in_maps = _host_prep(np.asarray(x, np.float32), np.asarray(conv_w),
                         np.asarray(bn_gamma), np.asarray(bn_beta),
                         np.asarray(bn_mean), np.asarray(bn_var),
                         precision=PRECISION)
    if "nc" not in _CACHED:
        _CACHED["nc"] = _build_bass(precision=PRECISION)
    res = run_bass_kernel_spmd(_CACHED["nc"], in_maps, list(range(NCORES)))
    out = np.empty((B, O, N), np.float32)
    for core in range(NCORES):
        b, h = core // 2, core % 2
        out[b, :, h * HALF:(h + 1) * HALF] = res.results[core]["y"].T
    return out


# revision 16
# speedup vs baseline: 1.6800x; 1.6692x over previous
"""Trainium2 Bass kernel for the DGCNN-style EdgeConv layer (KNN graph +
1x1 conv + BN + LeakyReLU + max over neighbors).

Math: for each batch b and point n,
  out[b, :, n] = lrelu( max_{m in KNN16(n)} u[m, :] + v[n, :] )
where u[m, :] = inv * (W1 @ x_m),  v[n, :] = inv * ((W2 - W1) @ x_n) + c,
W = [W1 W2] the 1x1-conv weight, inv/c the folded BN affine. LeakyReLU
commutes with the max since it is monotone, and the conv of
[nbr - ctr, ctr] splits into the u/v terms above, so only the KNN
selection and a 16-row gather+max remain data-dependent.

Device pipeline per core (one batch-half, 4096 query rows):
  - PE computes score chunks s[n, m] = 2<x_n, x_m> - |x_m|^2 via an
    augmented-contraction matmul (65th row carries -|x_m|^2).
  - ACT evicts PSUM -> SBUF.
  - DVE per 512-chunk: max8 (top-8 values) + max_index (their in-chunk
    positions) -> 128 candidates/row; candidates are packed as
    round(4*s) + (8191 - global_pos) * 2^-13 (exact in fp32) so one
    max8 chain on the packed array yields the top-17 values AND their
    global indices with jax-compatible tie-breaking. Slot 0 is provably
    the diagonal (self-match), which is dropped -> 16 neighbor indices.
  - Neighbor u-rows are fetched with a gpsimd dma_gather (512B rows)
    and max-reduced pairwise; v is added, LeakyReLU applied, and the
    [n, o] block DMA'd out (final transpose to [o, n] happens on host).
"""

import numpy as np

B, C, N, O, K = 4, 64, 8192, 128, 16
NCORES = 8
HALF = N // 2        # query rows per core
NBLK = HALF // 128   # 32 row blocks
CHUNK = 512
NCHUNK = N // CHUNK  # 16
BN_EPS = 1e-5
LRELU = 0.2
CBIG = 12582912.0    # 1.5 * 2^23: fp32 add forces round-to-integer
NEG = -3.0e38

_CACHED = {}


def _build_bass(finalize=True, stage='full', precision='f32r'):
    import concourse.bacc as bacc
    import concourse.tile as tile
    from concourse import mybir

    f32 = mybir.dt.float32
    i16 = mybir.dt.int16
    u16 = mybir.dt.uint16
    f32r = mybir.dt.float32r
    Alu = mybir.AluOpType

    AUG = 65 if precision == 'fp32' else 66
    mmdt = f32 if precision == 'fp32' else f32r
    nc = bacc.Bacc(num_swdge_queues=4)
    lhs_d = nc.declare_dram_parameter("lhs_aug", [AUG, HALF], mmdt, isOutput=False)
    rhs_d = nc.declare_dram_parameter("rhs_aug", [AUG, N], mmdt, isOutput=False)
    u_d = nc.declare_dram_parameter("u", [N, O], f32, isOutput=False)
    v_d = nc.declare_dram_parameter("v", [128, HALF], f32, isOutput=False)
    cb_d = nc.declare_dram_parameter("cbase", [128, 128], f32, isOutput=False)
    y_d = nc.declare_dram_parameter("y", [HALF, O], f32, isOutput=True)
    bounce_d = nc.dram_tensor("bounce", [NBLK, 16, 128], i16)

    with tile.TileContext(nc) as tc:
        with tc.tile_pool(name="const", bufs=1) as constp, \
             tc.tile_pool(name="score", bufs=2) as scorep, \
             tc.tile_pool(name="psum", bufs=8, space="PSUM") as psump, \
             tc.tile_pool(name="cand", bufs=2) as candp, \
             tc.tile_pool(name="small", bufs=4) as smallp, \
             tc.tile_pool(name="gather", bufs=4) as gatherp:

            lhs = constp.tile([AUG, HALF], mmdt)
            nc.sync.dma_start(lhs[:], lhs_d[:])
            rhs = constp.tile([AUG, N], mmdt)
            nc.sync.dma_start(rhs[:], rhs_d[:])
            v_sb = constp.tile([128, HALF], f32)
            nc.sync.dma_start(v_sb[:], v_d[:])
            cbase = constp.tile([128, 128], f32)
            nc.sync.dma_start(cbase[:], cb_d[:])

            pending = []

            def phaseB(blk, ug):
                # max over the 16 gathered u-rows (pairwise tree)
                r1 = gatherp.tile([128, 8 * O], f32, tag="r1")
                nc.vector.tensor_max(r1[:], ug[:, 0:8 * O], ug[:, 8 * O:16 * O])
                r2 = gatherp.tile([128, 4 * O], f32, tag="r2")
                nc.vector.tensor_max(r2[:], r1[:, 0:4 * O], r1[:, 4 * O:8 * O])
                r3 = gatherp.tile([128, 2 * O], f32, tag="r3")
                nc.vector.tensor_max(r3[:], r2[:, 0:2 * O], r2[:, 2 * O:4 * O])
                s16 = gatherp.tile([128, O], f32, tag="s16")
                nc.vector.tensor_max(s16[:], r3[:, 0:O], r3[:, O:2 * O])

                # y = lrelu(s16 + v)
                y1 = gatherp.tile([128, O], f32, tag="y1")
                nc.vector.tensor_add(y1[:], s16[:],
                                     v_sb[:, blk * 128:(blk + 1) * 128])
                yb = gatherp.tile([128, O], f32, tag="yb")
                nc.vector.scalar_tensor_tensor(yb[:], y1[:], LRELU, y1[:],
                                               Alu.mult, Alu.max)
                nc.sync.dma_start(y_d[blk * 128:(blk + 1) * 128, :], yb[:])

            for blk in range(NBLK):
                s_sb = scorep.tile([128, N], f32, tag="s")
                cand = candp.tile([128, 128], f32, tag="cv")
                cpos = candp.tile([128, 128], u16, tag="cp")
                lhsT = lhs[:, blk * 128:(blk + 1) * 128]
                for c in range(NCHUNK):
                    ps = psump.tile([128, CHUNK], f32, tag="ps")
                    nc.tensor.matmul(ps[:], lhsT,
                                     rhs[:, c * CHUNK:(c + 1) * CHUNK],
                                     start=True, stop=True)
                    sc = s_sb[:, c * CHUNK:(c + 1) * CHUNK]
                    nc.scalar.copy(sc, ps[:])
                    nc.vector.max(cand[:, c * 8:(c + 1) * 8], sc)
                    nc.vector.max_index(cpos[:, c * 8:(c + 1) * 8],
                                        cand[:, c * 8:(c + 1) * 8], sc)

                # gcand[slot] = global index of candidate slot
                cp_f = smallp.tile([128, 128], f32, tag="cpf")
                nc.vector.tensor_copy(cp_f[:], cpos[:])
                gcand = smallp.tile([128, 128], f32, tag="gcand")
                nc.vector.tensor_add(gcand[:], cp_f[:], cbase[:])

                # top-17 chain on a copy of the exact candidate values
                candc = smallp.tile([128, 128], f32, tag="candc")
                nc.vector.tensor_copy(candc[:], cand[:])
                w = smallp.tile([128, 24], f32, tag="w")
                nc.vector.max(w[:, 0:8], candc[:])
                nc.vector.match_replace(candc[:], w[:, 0:8], candc[:], NEG)
                nc.vector.max(w[:, 8:16], candc[:])
                nc.vector.match_replace(candc[:], w[:, 8:16], candc[:], NEG)
                nc.vector.max(w[:, 16:24], candc[:])

                # extract winners' global indices: for rank j (1..16, rank 0 is
                # the diagonal self-match), gm_j = sum(gcand * (cand == w_j))
                gm = smallp.tile([128, K], f32, tag="gm")
                dummy = smallp.tile([128, 128], f32, tag="dummy")
                for j in range(1, 17):
                    nc.vector.scalar_tensor_tensor(
                        dummy[:], cand[:], w[:, j:j + 1], gcand[:],
                        Alu.is_equal, Alu.mult,
                        accum_out=gm[:, j - 1:j])

                # neighbor indices -> int16, bounce via DRAM to the k-major
                # 16-partition-wrapped layout dma_gather expects
                mi = smallp.tile([128, K], i16, tag="mi")
                nc.vector.tensor_copy(mi[:], gm[:])
                # bounce out writes the 16-partition-wrapped layout directly:
                # wrapped[q, 8k + nh] = m[16 nh + q, k]
                nc.sync.dma_start(
                    bounce_d[blk].rearrange("q (k nh) -> nh q k", nh=8), mi[:])
                idxsb = smallp.tile([128, 128], i16, tag="idx")
                if stage != 'nobounce':
                    bview8 = bounce_d[blk].unsqueeze(0).to_broadcast([8, 16, 128])
                    nc.sync.dma_start(idxsb[:], bview8)
                else:
                    nc.vector.memset(idxsb[:], 0)

                # dma_gather is limited to 1024 indices per call -> 2 calls
                ug = gatherp.tile([128, K * O], f32, tag="ug")
                if stage in ('full', 'nobounce'):
                    for half in range(2):
                        nc.gpsimd.dma_gather(
                            ug[:, half * 8 * O:(half + 1) * 8 * O].rearrange(
                                "p (k o) -> p k o", o=O),
                            u_d[:],
                            idxsb[:, half * 64:(half + 1) * 64],
                            num_idxs=8 * 128,
                            num_idxs_reg=8 * 128,
                            elem_size=O,
                            queue_num=blk % 4,
                        )
                else:
                    nc.vector.memset(ug[:], 0.0)

                pending.append((blk, ug))
                if len(pending) > 2:
                    b2, ug2 = pending.pop(0)
                    phaseB(b2, ug2)
            for b2, ug2 in pending:
                phaseB(b2, ug2)

    if finalize:
        nc.finalize()
    return nc


def _rne11(a):
    b = np.ascontiguousarray(a, np.float32).view(np.uint32).copy()
    lsb = (b >> 12) & 1
    b = ((b + 2047 + lsb) >> 12) << 12
    return b.view(np.float32)


def _host_prep(x, conv_w, bn_gamma, bn_beta, bn_mean, bn_var, precision='f32r'):
    f32 = np.float32
    inv = (bn_gamma / np.sqrt(bn_var + BN_EPS)).astype(f32)
    cvec = (bn_beta - bn_mean * inv).astype(f32)
    W1 = conv_w[:, :C].astype(f32)
    W2 = conv_w[:, C:].astype(f32)
    cbase = np.broadcast_to(
        (CHUNK * (np.arange(128) // 8)).astype(f32)[None, :],
        (128, 128)).astype(f32).copy()
    in_maps = []
    for core in range(NCORES):
        b, h = core // 2, core % 2
        xb = np.asarray(x[b], dtype=f32)                       # [C, N]
        sq = (xb * xb).sum(0, dtype=f32)
        if precision == 'fp32':
            lhs_aug = np.concatenate(
                [2.0 * xb[:, h * HALF:(h + 1) * HALF], np.ones((1, HALF), f32)], 0)
            rhs_aug = np.concatenate([xb, -sq[None, :]], 0)
        else:
            # consistent rounded-cloud geometry: selection = exact KNN of the
            # f32r-rounded points; norms split hi/lo so they stay f32r-exact
            xr = _rne11(xb)
            sqr = (xr * xr).sum(0, dtype=np.float64).astype(f32)
            sq_hi = _rne11(sqr)
            sq_lo = _rne11(sqr - sq_hi)
            ones = np.ones((1, HALF), f32)
            lhs_aug = np.concatenate(
                [2.0 * xr[:, h * HALF:(h + 1) * HALF], ones, ones], 0)
            rhs_aug = np.concatenate([xr, -sq_hi[None, :], -sq_lo[None, :]], 0)
        u = (xb.T @ W1.T) * inv[None, :]                       # [N, O]
        vfull = (xb.T @ (W2 - W1).T) * inv[None, :] + cvec[None, :]
        vh = vfull[h * HALF:(h + 1) * HALF]                    # [HALF, O]
        v_sb = vh.reshape(NBLK, 128, O).transpose(1, 0, 2).reshape(128, HALF)
        in_maps.append({
            "lhs_aug": np.ascontiguousarray(lhs_aug, dtype=f32),
            "rhs_aug": np.ascontiguousarray(rhs_aug, dtype=f32),
            "u": np.ascontiguousarray(u, dtype=f32),
            "v": np.ascontiguousarray(v_sb, dtype=f32),
            "cbase": cbase,
        })
    return in_maps


PRECISION = 'f32r'


def kernel(x, conv_w, bn_gamma, bn_beta, bn_mean, bn_var):
    from concourse.bass_utils import run_bass_kernel_spmd

    x = np.asarray(x)
    in_maps = _host_prep(np.asarray(x, np.float32), np.asarray(conv_w),
                         np.asarray(bn_gamma), np.asarray(bn_beta),
                         np.asarray(bn_mean), np.asarray(bn_var),
                         precision=PRECISION)
    if "nc" not in _CACHED:
        _CACHED["nc"] = _build_bass(precision=PRECISION)
    res = run_bass_kernel_spmd(_CACHED["nc"], in_maps, list(range(NCORES)))
    out = np.empty((B, O, N), np.float32)
    for core in range(NCORES):
        b, h = core // 2, core % 2
        out[b, :, h * HALF:(h + 1) * HALF] = res.results[core]["y"].T
    return out


# revision 19
# speedup vs baseline: 1.7447x; 1.0385x over previous
"""Trainium2 Bass kernel for the DGCNN-style EdgeConv layer (KNN graph +
1x1 conv + BN + LeakyReLU + max over neighbors).

Math: for each batch b and point n,
  out[b, :, n] = lrelu( max_{m in KNN16(n)} u[m, :] + v[n, :] )
where u[m, :] = inv * (W1 @ x_m),  v[n, :] = inv * ((W2 - W1) @ x_n) + c,
W = [W1 W2] the 1x1-conv weight, inv/c the folded BN affine. LeakyReLU
commutes with the max (monotone), and the conv of [nbr - ctr, ctr]
splits into the u/v terms, so only the KNN selection and a 16-row
gather+max remain data-dependent.

Selection scores s[n, m] = 2<x_n, x_m> - |x_m|^2 are computed exactly
for the fp16-pair-rounded point cloud: x ~ xh + xl (two fp16 halves,
22-bit mantissa, 2^-22 relative residual). Per PSUM chunk the -|x_m|^2
row is prefilled by the Scalar engine, then two fp16 matmuls accumulate
  [ah; al]^T [bh; bl] = ah.bh + al.bl
  [al; ah]^T [bh; bl] = al.bh + ah.bl
which together give the exact pair-product. Since 2x rounds to exactly
twice the rounding of x, the scores are the exact KNN geometry of the
rounded cloud - selection matches fp32 reference selection (residual
2^-22 is far below typical 16th/17th-neighbor gaps).

Device pipeline per core (one batch-half, 4096 query rows):
  - per 1024-wide chunk: ACT prefill + 4 matmuls -> PSUM scores
  - DVE: max8 + max_index per chunk -> 64 candidates/row (top-8 per
    chunk provably covers the row top-17 unless one chunk holds >8 of
    them - statistically negligible)
  - top-17 chain (3x max8 + 2x match_replace) on exact candidate
    values; winner global indices extracted collision-free via
    per-rank is_equal/accumulate against the candidate array
  - neighbor u-rows fetched with gpsimd dma_gather (512B rows, 4 SWDGE
    queues), max-reduced pairwise, v added, LeakyReLU, block DMA'd out
    ([n, o] layout; final transpose to [o, n] on host).
The per-block tail (reduce+output) is software-pipelined two blocks
behind so gather latency hides under the next blocks' DVE work.
"""

import numpy as np

B, C, N, O, K = 4, 64, 8192, 128, 16
NCORES = 8
HALF = N // 2        # query rows per core
NBLK = HALF // 128   # 32 row blocks
CHUNK = 1024
NCHUNK = N // CHUNK  # 8
NSLOT = NCHUNK * 8   # 64 candidate slots per row
BN_EPS = 1e-5
LRELU = 0.2
NEG = -3.0e38

_CACHED = {}


def _build_bass(finalize=True):
    import concourse.bacc as bacc
    import concourse.tile as tile
    from concourse import mybir

    f32 = mybir.dt.float32
    f16 = mybir.dt.float16
    i16 = mybir.dt.int16
    u16 = mybir.dt.uint16
    Alu = mybir.AluOpType

    nc = bacc.Bacc(num_swdge_queues=4)
    lhs_d = nc.declare_dram_parameter("lhs_hl", [128, HALF], f16, isOutput=False)
    lhs2_d = nc.declare_dram_parameter("lhs_lh", [128, HALF], f16, isOutput=False)
    rhs_d = nc.declare_dram_parameter("rhs_hl", [128, N], f16, isOutput=False)
    nsq_d = nc.declare_dram_parameter("negsq", [1, N], f32, isOutput=False)
    u_d = nc.declare_dram_parameter("u", [N, O], f32, isOutput=False)
    v_d = nc.declare_dram_parameter("v", [128, HALF], f32, isOutput=False)
    cb_d = nc.declare_dram_parameter("cbase", [128, NSLOT], f32, isOutput=False)
    y_d = nc.declare_dram_parameter("y", [HALF, O], f32, isOutput=True)
    bounce_d = nc.declare_dram_parameter("bounce", [NBLK, 16, 128], i16, isOutput=True)

    with tile.TileContext(nc) as tc:
        with tc.tile_pool(name="const", bufs=1) as constp, \
             tc.tile_pool(name="psum", bufs=4, space="PSUM") as psump, \
             tc.tile_pool(name="cand", bufs=2) as candp, \
             tc.tile_pool(name="small", bufs=4) as smallp, \
             tc.tile_pool(name="gather", bufs=4) as gatherp:

            lhs = constp.tile([128, HALF], f16)
            nc.sync.dma_start(lhs[:], lhs_d[:])
            lhs2 = constp.tile([128, HALF], f16)
            nc.sync.dma_start(lhs2[:], lhs2_d[:])
            rhs = constp.tile([128, N], f16)
            nc.sync.dma_start(rhs[:], rhs_d[:])
            nsq = constp.tile([128, N], f32)
            nc.sync.dma_start(nsq[:], nsq_d[:].to_broadcast([128, N]))
            v_sb = constp.tile([128, HALF], f32)
            nc.sync.dma_start(v_sb[:], v_d[:])
            cbase = constp.tile([128, NSLOT], f32)
            nc.sync.dma_start(cbase[:], cb_d[:])

            pending = []

            def phaseB(blk, ug):
                # max over the 16 gathered u-rows. The first-stage ops each
                # read only ONE dma_gather's output region: Tile tracks each
                # gather's completion on a different DMA-SW semaphore and a
                # consumer spanning both can lose one wait (observed race).
                ra = gatherp.tile([128, 4 * O], f32, tag="ra")
                nc.vector.tensor_max(ra[:], ug[:, 0:4 * O], ug[:, 4 * O:8 * O])
                rb = gatherp.tile([128, 4 * O], f32, tag="rb")
                nc.vector.tensor_max(rb[:], ug[:, 8 * O:12 * O], ug[:, 12 * O:16 * O])
                r2 = gatherp.tile([128, 4 * O], f32, tag="r2")
                nc.vector.tensor_max(r2[:], ra[:], rb[:])
                r3 = gatherp.tile([128, 2 * O], f32, tag="r3")
                nc.vector.tensor_max(r3[:], r2[:, 0:2 * O], r2[:, 2 * O:4 * O])
                s16 = gatherp.tile([128, O], f32, tag="s16")
                nc.vector.tensor_max(s16[:], r3[:, 0:O], r3[:, O:2 * O])

                # y = lrelu(s16 + v)
                y1 = gatherp.tile([128, O], f32, tag="y1")
                nc.vector.tensor_add(y1[:], s16[:],
                                     v_sb[:, blk * 128:(blk + 1) * 128])
                yb = gatherp.tile([128, O], f32, tag="yb")
                nc.vector.scalar_tensor_tensor(yb[:], y1[:], LRELU, y1[:],
                                               Alu.mult, Alu.max)
                nc.sync.dma_start(y_d[blk * 128:(blk + 1) * 128, :], yb[:])

            for blk in range(NBLK):
                cand = candp.tile([128, NSLOT], f32, tag="cv")
                cpos = candp.tile([128, NSLOT], u16, tag="cp")
                lhsT = lhs[:, blk * 128:(blk + 1) * 128]
                lhsT2 = lhs2[:, blk * 128:(blk + 1) * 128]
                for c in range(NCHUNK):
                    ps = psump.tile([128, CHUNK], f32, tag="ps")
                    # prefill -|x_m|^2, then accumulate the two fp16 products
                    nc.scalar.copy(ps[:], nsq[:, c * CHUNK:(c + 1) * CHUNK])
                    for h in range(2):
                        rcols = rhs[:, c * CHUNK + h * 512:c * CHUNK + (h + 1) * 512]
                        pcols = ps[:, h * 512:(h + 1) * 512]
                        nc.tensor.matmul(pcols, lhsT, rcols, start=False,
                                         stop=False, skip_group_check=True)
                        nc.tensor.matmul(pcols, lhsT2, rcols, start=False,
                                         stop=True, skip_group_check=True)
                    nc.vector.max(cand[:, c * 8:(c + 1) * 8], ps[:])
                    nc.vector.max_index(cpos[:, c * 8:(c + 1) * 8],
                                        cand[:, c * 8:(c + 1) * 8], ps[:])

                # gcand[slot] = global index of candidate slot (ACT casts)
                cp_f = smallp.tile([128, NSLOT], f32, tag="cpf")
                nc.scalar.copy(cp_f[:], cpos[:])
                gcand = smallp.tile([128, NSLOT], f32, tag="gcand")
                nc.vector.tensor_add(gcand[:], cp_f[:], cbase[:])

                # top-17 chain on a copy of the exact candidate values
                candc = smallp.tile([128, NSLOT], f32, tag="candc")
                nc.scalar.copy(candc[:], cand[:])
                w = smallp.tile([128, 24], f32, tag="w")
                nc.vector.max(w[:, 0:8], candc[:])
                nc.vector.match_replace(candc[:], w[:, 0:8], candc[:], NEG)
                nc.vector.max(w[:, 8:16], candc[:])
                nc.vector.match_replace(candc[:], w[:, 8:16], candc[:], NEG)
                nc.vector.max(w[:, 16:24], candc[:])

                # extract winners' global indices: for rank j (1..16, rank 0
                # is the diagonal self-match), gm_j = sum(gcand*(cand == w_j))
                gm = smallp.tile([128, K], f32, tag="gm")
                dummy = smallp.tile([128, NSLOT], f32, tag="dummy")
                for j in range(1, 17):
                    nc.vector.scalar_tensor_tensor(
                        dummy[:], cand[:], w[:, j:j + 1], gcand[:],
                        Alu.is_equal, Alu.mult,
                        accum_out=gm[:, j - 1:j])

                # indices -> int16 (ACT cast), bounce to the 16-partition-
                # wrapped k-major layout dma_gather expects:
                # wrapped[q, 8k + nh] = m[16 nh + q, k]
                mi = smallp.tile([128, K], i16, tag="mi")
                nc.scalar.copy(mi[:], gm[:])
                nc.sync.dma_start(
                    bounce_d[blk].rearrange("q (k nh) -> nh q k", nh=8), mi[:])
                idxsb = smallp.tile([128, 128], i16, tag="idx")
                nc.sync.dma_start(
                    idxsb[:], bounce_d[blk].unsqueeze(0).to_broadcast([8, 16, 128]))

                # dma_gather is limited to 1024 indices per call -> 2 calls
                ug = gatherp.tile([128, K * O], f32, tag="ug")
                for half in range(2):
                    nc.gpsimd.dma_gather(
                        ug[:, half * 8 * O:(half + 1) * 8 * O].rearrange(
                            "p (k o) -> p k o", o=O),
                        u_d[:],
                        idxsb[:, half * 64:(half + 1) * 64],
                        num_idxs=8 * 128,
                        num_idxs_reg=8 * 128,
                        elem_size=O,
                        queue_num=blk % 4,
                    )

                pending.append((blk, ug))
                if len(pending) > 2:
                    b2, ug2 = pending.pop(0)
                    phaseB(b2, ug2)
            for b2, ug2 in pending:
                phaseB(b2, ug2)

    if finalize:
        nc.finalize()
    return nc


def _host_prep(x, conv_w, bn_gamma, bn_beta, bn_mean, bn_var):
    f32 = np.float32
    inv = (bn_gamma / np.sqrt(bn_var + BN_EPS)).astype(f32)
    cvec = (bn_beta - bn_mean * inv).astype(f32)
    W1 = conv_w[:, :C].astype(f32)
    W2 = conv_w[:, C:].astype(f32)
    cbase = np.broadcast_to(
        (CHUNK * (np.arange(NSLOT) // 8)).astype(f32)[None, :],
        (128, NSLOT)).astype(f32).copy()
    in_maps = []
    for core in range(NCORES):
        b, h = core // 2, core % 2
        xb = np.asarray(x[b], dtype=f32)                       # [C, N]
        # fp16-pair split of the cloud; 2x rounds to exactly 2*rounded(x)
        bh = xb.astype(np.float16)
        bl = (xb - bh.astype(f32)).astype(np.float16)
        xe = bh.astype(f32) + bl.astype(f32)                   # effective cloud
        sq = (xe.astype(np.float64) ** 2).sum(0).astype(f32)
        ah, al = 2.0 * bh, 2.0 * bl                            # exact x2
        hcols = slice(h * HALF, (h + 1) * HALF)
        lhs_hl = np.concatenate([ah[:, hcols], al[:, hcols]], 0)
        lhs_lh = np.concatenate([al[:, hcols], ah[:, hcols]], 0)
        rhs_hl = np.concatenate([bh, bl], 0)
        u = (xb.T @ W1.T) * inv[None, :]                       # [N, O]
        vfull = (xb.T @ (W2 - W1).T) * inv[None, :] + cvec[None, :]
        vh = vfull[hcols]                                      # [HALF, O]
        v_sb = vh.reshape(NBLK, 128, O).transpose(1, 0, 2).reshape(128, HALF)
        in_maps.append({
            "lhs_hl": np.ascontiguousarray(lhs_hl, dtype=np.float16),
            "lhs_lh": np.ascontiguousarray(lhs_lh, dtype=np.float16),
            "rhs_hl": np.ascontiguousarray(rhs_hl, dtype=np.float16),
            "negsq": np.ascontiguousarray(-sq[None, :], dtype=f32),
            "u": np.ascontiguousarray(u, dtype=f32),
            "v": np.ascontiguousarray(v_sb, dtype=f32),
            "cbase": cbase,
        })
    return in_maps


def kernel(x, conv_w, bn_gamma, bn_beta, bn_mean, bn_var):
    from concourse.bass_utils import run_bass_kernel_spmd

    x = np.asarray(x)
    in_maps = _host_prep(np.asarray(x, np.float32), np.asarray(conv_w),
                         np.asarray(bn_gamma), np.asarray(bn_beta),
                         np.asarray(bn_mean), np.asarray(bn_var))
    if "nc" not in _CACHED:
        _CACHED["nc"] = _build_bass()
    res = run_bass_kernel_spmd(_CACHED["nc"], in_maps, list(range(NCORES)))
    out = np.empty((B, O, N), np.float32)
    for core in range(NCORES):
        b, h = core // 2, core % 2
        out[b, :, h * HALF:(h + 1) * HALF] = res.results[core]["y"].T
    return out


# revision 20
# speedup vs baseline: 2.4053x; 1.3786x over previous
"""Trainium2 Bass kernel for the DGCNN-style EdgeConv layer (KNN graph +
1x1 conv + BN + LeakyReLU + max over neighbors).

Math: for each batch b and point n,
  out[b, :, n] = lrelu( max_{m in KNN16(n)} u[m, :] + v[n, :] )
where u[m, :] = inv * (W1 @ x_m),  v[n, :] = inv * ((W2 - W1) @ x_n) + c,
W = [W1 W2] the 1x1-conv weight, inv/c the folded BN affine. LeakyReLU
commutes with the max (monotone), and the conv of [nbr - ctr, ctr]
splits into the u/v terms, so only the KNN selection and a 16-row
gather+max remain data-dependent.

Selection scores s[n, m] = 2<x_n, x_m> - |x_m|^2 are computed exactly
for the fp16-pair-rounded point cloud: x ~ xh + xl (two fp16 halves,
22-bit mantissa, 2^-22 relative residual). Per PSUM chunk the -|x_m|^2
row is prefilled by the Scalar engine, then two fp16 matmuls accumulate
  [ah; al]^T [bh; bl] = ah.bh + al.bl
  [al; ah]^T [bh; bl] = al.bh + ah.bl
which together give the exact pair-product. Since 2x rounds to exactly
twice the rounding of x, the scores are the exact KNN geometry of the
rounded cloud - selection matches fp32 reference selection (residual
2^-22 is far below typical 16th/17th-neighbor gaps).

Device pipeline per core (one batch-half, 4096 query rows):
  - per 1024-wide chunk: ACT prefill + 4 matmuls -> PSUM scores
  - DVE: max8 + max_index per chunk -> 64 candidates/row (top-8 per
    chunk provably covers the row top-17 unless one chunk holds >8 of
    them - statistically negligible)
  - top-17 chain (3x max8 + 2x match_replace) on exact candidate
    values; winner global indices extracted collision-free via
    per-rank is_equal/accumulate against the candidate array
  - neighbor u-rows fetched with gpsimd dma_gather (512B rows, 4 SWDGE
    queues), max-reduced pairwise, v added, LeakyReLU, block DMA'd out
    ([n, o] layout; final transpose to [o, n] on host).
The per-block tail (reduce+output) is software-pipelined two blocks
behind so gather latency hides under the next blocks' DVE work.
"""

import numpy as np

B, C, N, O, K = 4, 64, 8192, 128, 16
NCORES = 8
HALF = N // 2        # query rows per core
NBLK = HALF // 128   # 32 row blocks
CHUNK = 1024
NCHUNK = N // CHUNK  # 8
NSLOT = NCHUNK * 8   # 64 candidate slots per row
BN_EPS = 1e-5
LRELU = 0.2
NEG = -3.0e38

_CACHED = {}


def _build_bass(finalize=True):
    import concourse.bacc as bacc
    import concourse.tile as tile
    from concourse import mybir

    f32 = mybir.dt.float32
    f16 = mybir.dt.float16
    i16 = mybir.dt.int16
    u16 = mybir.dt.uint16
    Alu = mybir.AluOpType

    nc = bacc.Bacc(num_swdge_queues=4)
    lhs_d = nc.declare_dram_parameter("lhs_hl", [128, HALF], f16, isOutput=False)
    lhs2_d = nc.declare_dram_parameter("lhs_lh", [128, HALF], f16, isOutput=False)
    rhs_d = nc.declare_dram_parameter("rhs_hl", [128, N], f16, isOutput=False)
    nsq_d = nc.declare_dram_parameter("negsq", [1, N], f32, isOutput=False)
    u_d = nc.declare_dram_parameter("u", [N, O], f32, isOutput=False)
    v_d = nc.declare_dram_parameter("v", [128, HALF], f32, isOutput=False)
    cb_d = nc.declare_dram_parameter("cbase", [128, NSLOT], f32, isOutput=False)
    y_d = nc.declare_dram_parameter("y", [HALF, O], f32, isOutput=True)
    bounce_d = nc.declare_dram_parameter("bounce", [NBLK, 16, 128], i16, isOutput=True)

    with tile.TileContext(nc) as tc:
        with tc.tile_pool(name="const", bufs=1) as constp, \
             tc.tile_pool(name="psum", bufs=4, space="PSUM") as psump, \
             tc.tile_pool(name="cand", bufs=3) as candp, \
             tc.tile_pool(name="small", bufs=6) as smallp, \
             tc.tile_pool(name="gather", bufs=6) as gatherp:

            lhs = constp.tile([128, HALF], f16)
            nc.sync.dma_start(lhs[:], lhs_d[:])
            lhs2 = constp.tile([128, HALF], f16)
            nc.sync.dma_start(lhs2[:], lhs2_d[:])
            rhs = constp.tile([128, N], f16)
            nc.sync.dma_start(rhs[:], rhs_d[:])
            nsq = constp.tile([128, N], f32)
            nc.sync.dma_start(nsq[:], nsq_d[:].to_broadcast([128, N]))
            v_sb = constp.tile([128, HALF], f32)
            nc.sync.dma_start(v_sb[:], v_d[:])
            cbase = constp.tile([128, NSLOT], f32)
            nc.sync.dma_start(cbase[:], cb_d[:])

            pending = []

            def phaseB(blk, ug):
                # max over the 16 gathered u-rows. The first-stage ops each
                # read only ONE dma_gather's output region: Tile tracks each
                # gather's completion on a different DMA-SW semaphore and a
                # consumer spanning both can lose one wait (observed race).
                ra = gatherp.tile([128, 4 * O], f32, tag="ra")
                nc.vector.tensor_max(ra[:], ug[:, 0:4 * O], ug[:, 4 * O:8 * O])
                rb = gatherp.tile([128, 4 * O], f32, tag="rb")
                nc.vector.tensor_max(rb[:], ug[:, 8 * O:12 * O], ug[:, 12 * O:16 * O])
                r2 = gatherp.tile([128, 4 * O], f32, tag="r2")
                nc.vector.tensor_max(r2[:], ra[:], rb[:])
                r3 = gatherp.tile([128, 2 * O], f32, tag="r3")
                nc.vector.tensor_max(r3[:], r2[:, 0:2 * O], r2[:, 2 * O:4 * O])
                s16 = gatherp.tile([128, O], f32, tag="s16")
                nc.vector.tensor_max(s16[:], r3[:, 0:O], r3[:, O:2 * O])

                # y = lrelu(s16 + v)
                y1 = gatherp.tile([128, O], f32, tag="y1")
                nc.vector.tensor_add(y1[:], s16[:],
                                     v_sb[:, blk * 128:(blk + 1) * 128])
                yb = gatherp.tile([128, O], f32, tag="yb")
                nc.vector.scalar_tensor_tensor(yb[:], y1[:], LRELU, y1[:],
                                               Alu.mult, Alu.max)
                nc.sync.dma_start(y_d[blk * 128:(blk + 1) * 128, :], yb[:])

            for blk in range(NBLK):
                cand = candp.tile([128, NSLOT], f32, tag="cv")
                cpos = candp.tile([128, NSLOT], u16, tag="cp")
                lhsT = lhs[:, blk * 128:(blk + 1) * 128]
                lhsT2 = lhs2[:, blk * 128:(blk + 1) * 128]
                for c in range(NCHUNK):
                    ps = psump.tile([128, CHUNK], f32, tag="ps")
                    # prefill -|x_m|^2, then accumulate the two fp16 products
                    nc.scalar.copy(ps[:], nsq[:, c * CHUNK:(c + 1) * CHUNK])
                    for h in range(2):
                        rcols = rhs[:, c * CHUNK + h * 512:c * CHUNK + (h + 1) * 512]
                        pcols = ps[:, h * 512:(h + 1) * 512]
                        nc.tensor.matmul(pcols, lhsT, rcols, start=False,
                                         stop=False, skip_group_check=True)
                        nc.tensor.matmul(pcols, lhsT2, rcols, start=False,
                                         stop=True, skip_group_check=True)
                    nc.vector.max(cand[:, c * 8:(c + 1) * 8], ps[:])
                    nc.vector.max_index(cpos[:, c * 8:(c + 1) * 8],
                                        cand[:, c * 8:(c + 1) * 8], ps[:])

                # gcand[slot] = global index of candidate slot (ACT casts)
                cp_f = smallp.tile([128, NSLOT], f32, tag="cpf")
                nc.scalar.copy(cp_f[:], cpos[:])
                gcand = smallp.tile([128, NSLOT], f32, tag="gcand")
                nc.vector.tensor_add(gcand[:], cp_f[:], cbase[:])

                # top-17 chain on a copy of the exact candidate values
                candc = smallp.tile([128, NSLOT], f32, tag="candc")
                nc.scalar.copy(candc[:], cand[:])
                w = smallp.tile([128, 24], f32, tag="w")
                nc.vector.max(w[:, 0:8], candc[:])
                nc.vector.match_replace(candc[:], w[:, 0:8], candc[:], NEG)
                nc.vector.max(w[:, 8:16], candc[:])
                nc.vector.match_replace(candc[:], w[:, 8:16], candc[:], NEG)
                nc.vector.max(w[:, 16:24], candc[:])

                # extract winners' global indices: for rank j (1..16, rank 0
                # is the diagonal self-match), gm_j = sum(gcand*(cand == w_j))
                gm = smallp.tile([128, K], f32, tag="gm")
                dummy = smallp.tile([128, NSLOT], f32, tag="dummy")
                for j in range(1, 17):
                    nc.vector.scalar_tensor_tensor(
                        dummy[:], cand[:], w[:, j:j + 1], gcand[:],
                        Alu.is_equal, Alu.mult,
                        accum_out=gm[:, j - 1:j])

                # indices -> int16 (ACT cast), bounce to the 16-partition-
                # wrapped k-major layout dma_gather expects:
                # wrapped[q, 8k + nh] = m[16 nh + q, k]
                mi = smallp.tile([128, K], i16, tag="mi")
                nc.scalar.copy(mi[:], gm[:])
                nc.sync.dma_start(
                    bounce_d[blk].rearrange("q (k nh) -> nh q k", nh=8), mi[:])
                idxsb = smallp.tile([128, 128], i16, tag="idx")
                nc.sync.dma_start(
                    idxsb[:], bounce_d[blk].unsqueeze(0).to_broadcast([8, 16, 128]))

                ug = gatherp.tile([128, K * O], f32, tag="ug")
                nc.gpsimd.dma_gather(
                    ug[:].rearrange("p (k o) -> p k o", o=O),
                    u_d[:],
                    idxsb[:],
                    num_idxs=K * 128,
                    num_idxs_reg=K * 128,
                    elem_size=O,
                    queue_num=blk % 4,
                    single_packet=False,
                )

                pending.append((blk, ug))
                if len(pending) > 3:
                    b2, ug2 = pending.pop(0)
                    phaseB(b2, ug2)
            for b2, ug2 in pending:
                phaseB(b2, ug2)

    if finalize:
        nc.finalize()
    return nc


def _host_prep(x, conv_w, bn_gamma, bn_beta, bn_mean, bn_var):
    f32 = np.float32
    inv = (bn_gamma / np.sqrt(bn_var + BN_EPS)).astype(f32)
    cvec = (bn_beta - bn_mean * inv).astype(f32)
    W1 = conv_w[:, :C].astype(f32)
    W2 = conv_w[:, C:].astype(f32)
    cbase = np.broadcast_to(
        (CHUNK * (np.arange(NSLOT) // 8)).astype(f32)[None, :],
        (128, NSLOT)).astype(f32).copy()
    in_maps = []
    for core in range(NCORES):
        b, h = core // 2, core % 2
        xb = np.asarray(x[b], dtype=f32)                       # [C, N]
        # fp16-pair split of the cloud; 2x rounds to exactly 2*rounded(x)
        bh = xb.astype(np.float16)
        bl = (xb - bh.astype(f32)).astype(np.float16)
        xe = bh.astype(f32) + bl.astype(f32)                   # effective cloud
        sq = (xe.astype(np.float64) ** 2).sum(0).astype(f32)
        ah, al = 2.0 * bh, 2.0 * bl                            # exact x2
        hcols = slice(h * HALF, (h + 1) * HALF)
        lhs_hl = np.concatenate([ah[:, hcols], al[:, hcols]], 0)
        lhs_lh = np.concatenate([al[:, hcols], ah[:, hcols]], 0)
        rhs_hl = np.concatenate([bh, bl], 0)
        u = (xb.T @ W1.T) * inv[None, :]                       # [N, O]
        vfull = (xb.T @ (W2 - W1).T) * inv[None, :] + cvec[None, :]
        vh = vfull[hcols]                                      # [HALF, O]
        v_sb = vh.reshape(NBLK, 128, O).transpose(1, 0, 2).reshape(128, HALF)
        in_maps.append({
            "lhs_hl": np.ascontiguousarray(lhs_hl, dtype=np.float16),
            "lhs_lh": np.ascontiguousarray(lhs_lh, dtype=np.float16),
            "rhs_hl": np.ascontiguousarray(rhs_hl, dtype=np.float16),
            "negsq": np.ascontiguousarray(-sq[None, :], dtype=f32),
            "u": np.ascontiguousarray(u, dtype=f32),
            "v": np.ascontiguousarray(v_sb, dtype=f32),
            "cbase": cbase,
        })
    return in_maps


def kernel(x, conv_w, bn_gamma, bn_beta, bn_mean, bn_var):
    from concourse.bass_utils import run_bass_kernel_spmd

    x = np.asarray(x)
    in_maps = _host_prep(np.asarray(x, np.float32), np.asarray(conv_w),
                         np.asarray(bn_gamma), np.asarray(bn_beta),
                         np.asarray(bn_mean), np.asarray(bn_var))
    if "nc" not in _CACHED:
        _CACHED["nc"] = _build_bass()
    res = run_bass_kernel_spmd(_CACHED["nc"], in_maps, list(range(NCORES)))
    out = np.empty((B, O, N), np.float32)
    for core in range(NCORES):
        b, h = core // 2, core % 2
        out[b, :, h * HALF:(h + 1) * HALF] = res.results[core]["y"].T
    return out


# revision 24
# speedup vs baseline: 2.4899x; 1.0352x over previous
"""Trainium2 Bass kernel for the DGCNN-style EdgeConv layer (KNN graph +
1x1 conv + BN + LeakyReLU + max over neighbors).

Math: for each batch b and point n,
  out[b, :, n] = lrelu( max_{m in KNN16(n)} u[m, :] + v[n, :] )
where u[m, :] = inv * (W1 @ x_m),  v[n, :] = inv * ((W2 - W1) @ x_n) + c,
W = [W1 W2] the 1x1-conv weight, inv/c the folded BN affine. LeakyReLU
commutes with the max (monotone), and the conv of [nbr - ctr, ctr]
splits into the u/v terms, so only the KNN selection and a 16-row
gather+max remain data-dependent.

Selection scores s[n, m] = 2<x_n, x_m> - |x_m|^2 are computed exactly
for the fp16-pair-rounded point cloud: x ~ xh + xl (two fp16 halves,
22-bit mantissa, 2^-22 relative residual). Per PSUM chunk the -|x_m|^2
row is prefilled by the Scalar engine, then two fp16 matmuls accumulate
  [ah; al]^T [bh; bl] = ah.bh + al.bl
  [al; ah]^T [bh; bl] = al.bh + ah.bl
which together give the exact pair-product. Since 2x rounds to exactly
twice the rounding of x, the scores are the exact KNN geometry of the
rounded cloud - selection matches fp32 reference selection (residual
2^-22 is far below typical 16th/17th-neighbor gaps).

Device pipeline per core (one batch-half, 4096 query rows):
  - per 1024-wide chunk: ACT prefill + 4 matmuls -> PSUM scores
  - DVE: max8 + max_index per chunk -> 64 candidates/row (top-8 per
    chunk provably covers the row top-17 unless one chunk holds >8 of
    them - statistically negligible)
  - top-17 chain (3x max8 + 2x match_replace) on exact candidate
    values; winner global indices extracted collision-free via
    per-rank is_equal/accumulate against the candidate array
  - neighbor u-rows fetched with gpsimd dma_gather (512B rows, 4 SWDGE
    queues), max-reduced pairwise, v added, LeakyReLU, block DMA'd out
    ([n, o] layout; final transpose to [o, n] on host).
The per-block tail (reduce+output) is software-pipelined two blocks
behind so gather latency hides under the next blocks' DVE work.
"""

import numpy as np

B, C, N, O, K = 4, 64, 8192, 128, 16
NCORES = 8
HALF = N // 2        # query rows per core
NBLK = HALF // 128   # 32 row blocks
CHUNK = 1024
NCHUNK = N // CHUNK  # 8
NSLOT = NCHUNK * 8   # 64 candidate slots per row
BN_EPS = 1e-5
LRELU = 0.2
NEG = -3.0e38

_CACHED = {}


def _build_bass(finalize=True):
    import concourse.bacc as bacc
    import concourse.tile as tile
    from concourse import mybir

    f32 = mybir.dt.float32
    f16 = mybir.dt.float16
    i16 = mybir.dt.int16
    u16 = mybir.dt.uint16
    Alu = mybir.AluOpType

    nc = bacc.Bacc(num_swdge_queues=4)
    lhs_d = nc.declare_dram_parameter("lhs_hl", [128, HALF], f16, isOutput=False)
    lhs2_d = nc.declare_dram_parameter("lhs_lh", [128, HALF], f16, isOutput=False)
    rhs_d = nc.declare_dram_parameter("rhs_hl", [128, N], f16, isOutput=False)
    nsq_d = nc.declare_dram_parameter("negsq", [1, N], f32, isOutput=False)
    u_d = nc.declare_dram_parameter("u", [N, O], f32, isOutput=False)
    v_d = nc.declare_dram_parameter("v", [128, HALF], f32, isOutput=False)
    cb_d = nc.declare_dram_parameter("cbase", [128, NSLOT], f32, isOutput=False)
    y_d = nc.declare_dram_parameter("y", [HALF, O], f32, isOutput=True)
    bounce_d = nc.dram_tensor("bounce", [NBLK, 16, 128], i16)

    with tile.TileContext(nc) as tc:
        with tc.tile_pool(name="const", bufs=1) as constp, \
             tc.tile_pool(name="psum", bufs=4, space="PSUM") as psump, \
             tc.tile_pool(name="cand", bufs=3) as candp, \
             tc.tile_pool(name="small", bufs=6) as smallp, \
             tc.tile_pool(name="gather", bufs=6) as gatherp:

            lhs = constp.tile([128, HALF], f16)
            nc.sync.dma_start(lhs[:], lhs_d[:])
            lhs2 = constp.tile([128, HALF], f16)
            nc.sync.dma_start(lhs2[:], lhs2_d[:])
            rhs = constp.tile([128, N], f16)
            nc.sync.dma_start(rhs[:], rhs_d[:])
            nsq = constp.tile([128, N], f32)
            nc.sync.dma_start(nsq[:], nsq_d[:].to_broadcast([128, N]))
            v_sb = constp.tile([128, HALF], f32)
            nc.sync.dma_start(v_sb[:], v_d[:])
            cbase = constp.tile([128, NSLOT], f32)
            nc.sync.dma_start(cbase[:], cb_d[:])

            pending = []

            def phaseB(blk, ug):
                # max over the 16 gathered u-rows. The first-stage ops each
                # read only ONE dma_gather's output region: Tile tracks each
                # gather's completion on a different DMA-SW semaphore and a
                # consumer spanning both can lose one wait (observed race).
                ra = gatherp.tile([128, 4 * O], f32, tag="ra")
                nc.vector.tensor_max(ra[:], ug[:, 0:4 * O], ug[:, 4 * O:8 * O])
                rb = gatherp.tile([128, 4 * O], f32, tag="rb")
                nc.vector.tensor_max(rb[:], ug[:, 8 * O:12 * O], ug[:, 12 * O:16 * O])
                r2 = gatherp.tile([128, 4 * O], f32, tag="r2")
                nc.vector.tensor_max(r2[:], ra[:], rb[:])
                r3 = gatherp.tile([128, 2 * O], f32, tag="r3")
                nc.vector.tensor_max(r3[:], r2[:, 0:2 * O], r2[:, 2 * O:4 * O])
                s16 = gatherp.tile([128, O], f32, tag="s16")
                nc.vector.tensor_max(s16[:], r3[:, 0:O], r3[:, O:2 * O])

                # y = lrelu(s16 + v)
                y1 = gatherp.tile([128, O], f32, tag="y1")
                nc.vector.tensor_add(y1[:], s16[:],
                                     v_sb[:, blk * 128:(blk + 1) * 128])
                yb = gatherp.tile([128, O], f32, tag="yb")
                nc.vector.scalar_tensor_tensor(yb[:], y1[:], LRELU, y1[:],
                                               Alu.mult, Alu.max)
                nc.sync.dma_start(y_d[blk * 128:(blk + 1) * 128, :], yb[:])

            for blk in range(NBLK):
                cand = candp.tile([128, NSLOT], f32, tag="cv")
                cpos = candp.tile([128, NSLOT], u16, tag="cp")
                lhsT = lhs[:, blk * 128:(blk + 1) * 128]
                lhsT2 = lhs2[:, blk * 128:(blk + 1) * 128]
                for c in range(NCHUNK):
                    ps = psump.tile([128, CHUNK], f32, tag="ps")
                    # prefill -|x_m|^2, then accumulate the two fp16 products
                    nc.scalar.copy(ps[:], nsq[:, c * CHUNK:(c + 1) * CHUNK])
                    for h in range(2):
                        rcols = rhs[:, c * CHUNK + h * 512:c * CHUNK + (h + 1) * 512]
                        pcols = ps[:, h * 512:(h + 1) * 512]
                        nc.tensor.matmul(pcols, lhsT, rcols, start=False,
                                         stop=False, skip_group_check=True)
                        nc.tensor.matmul(pcols, lhsT2, rcols, start=False,
                                         stop=True, skip_group_check=True)
                    nc.vector.max(cand[:, c * 8:(c + 1) * 8], ps[:])
                    nc.vector.max_index(cpos[:, c * 8:(c + 1) * 8],
                                        cand[:, c * 8:(c + 1) * 8], ps[:])

                # gcand[slot] = global index of candidate slot (ACT casts)
                cp_f = smallp.tile([128, NSLOT], f32, tag="cpf")
                nc.scalar.copy(cp_f[:], cpos[:])
                gcand = smallp.tile([128, NSLOT], f32, tag="gcand")
                nc.vector.tensor_add(gcand[:], cp_f[:], cbase[:])

                # top-17 chain on a copy of the exact candidate values
                candc = smallp.tile([128, NSLOT], f32, tag="candc")
                nc.scalar.copy(candc[:], cand[:])
                w = smallp.tile([128, 24], f32, tag="w")
                nc.vector.max(w[:, 0:8], candc[:])
                nc.vector.match_replace(candc[:], w[:, 0:8], candc[:], NEG)
                nc.vector.max(w[:, 8:16], candc[:])
                nc.vector.match_replace(candc[:], w[:, 8:16], candc[:], NEG)
                nc.vector.max(w[:, 16:24], candc[:])

                # extract winners' global indices: for rank j (1..16, rank 0
                # is the diagonal self-match), gm_j = sum(gcand*(cand == w_j))
                gm = smallp.tile([128, K], f32, tag="gm")
                dummy = smallp.tile([128, NSLOT], f32, tag="dummy")
                for j in range(1, 17):
                    nc.vector.scalar_tensor_tensor(
                        dummy[:], cand[:], w[:, j:j + 1], gcand[:],
                        Alu.is_equal, Alu.mult,
                        accum_out=gm[:, j - 1:j])

                # indices -> int16 (ACT cast), bounce to the 16-partition-
                # wrapped k-major layout dma_gather expects:
                # wrapped[q, 8k + nh] = m[16 nh + q, k]
                mi = smallp.tile([128, K], i16, tag="mi")
                nc.scalar.copy(mi[:], gm[:])
                nc.sync.dma_start(
                    bounce_d[blk].rearrange("q (k nh) -> nh q k", nh=8), mi[:])
                idxsb = smallp.tile([128, 128], i16, tag="idx")
                nc.sync.dma_start(
                    idxsb[:], bounce_d[blk].unsqueeze(0).to_broadcast([8, 16, 128]))

                ug = gatherp.tile([128, K * O], f32, tag="ug")
                nc.gpsimd.dma_gather(
                    ug[:].rearrange("p (k o) -> p k o", o=O),
                    u_d[:],
                    idxsb[:],
                    num_idxs=K * 128,
                    num_idxs_reg=K * 128,
                    elem_size=O,
                    queue_num=blk % 4,
                    single_packet=False,
                )

                pending.append((blk, ug))
                if len(pending) > 3:
                    b2, ug2 = pending.pop(0)
                    phaseB(b2, ug2)
            for b2, ug2 in pending:
                phaseB(b2, ug2)

    if finalize:
        nc.finalize()
    return nc


def _host_prep(x, conv_w, bn_gamma, bn_beta, bn_mean, bn_var):
    f32 = np.float32
    inv = (bn_gamma / np.sqrt(bn_var + BN_EPS)).astype(f32)
    cvec = (bn_beta - bn_mean * inv).astype(f32)
    W1 = conv_w[:, :C].astype(f32)
    W2 = conv_w[:, C:].astype(f32)
    cbase = np.broadcast_to(
        (CHUNK * (np.arange(NSLOT) // 8)).astype(f32)[None, :],
        (128, NSLOT)).astype(f32).copy()
    in_maps = []
    for core in range(NCORES):
        b, h = core // 2, core % 2
        xb = np.asarray(x[b], dtype=f32)                       # [C, N]
        # fp16-pair split of the cloud; 2x rounds to exactly 2*rounded(x)
        bh = xb.astype(np.float16)
        bl = (xb - bh.astype(f32)).astype(np.float16)
        xe = bh.astype(f32) + bl.astype(f32)                   # effective cloud
        sq = (xe.astype(np.float64) ** 2).sum(0).astype(f32)
        ah, al = 2.0 * bh, 2.0 * bl                            # exact x2
        hcols = slice(h * HALF, (h + 1) * HALF)
        lhs_hl = np.concatenate([ah[:, hcols], al[:, hcols]], 0)
        lhs_lh = np.concatenate([al[:, hcols], ah[:, hcols]], 0)
        rhs_hl = np.concatenate([bh, bl], 0)
        u = (xb.T @ W1.T) * inv[None, :]                       # [N, O]
        vfull = (xb.T @ (W2 - W1).T) * inv[None, :] + cvec[None, :]
        vh = vfull[hcols]                                      # [HALF, O]
        v_sb = vh.reshape(NBLK, 128, O).transpose(1, 0, 2).reshape(128, HALF)
        in_maps.append({
            "lhs_hl": np.ascontiguousarray(lhs_hl, dtype=np.float16),
            "lhs_lh": np.ascontiguousarray(lhs_lh, dtype=np.float16),
            "rhs_hl": np.ascontiguousarray(rhs_hl, dtype=np.float16),
            "negsq": np.ascontiguousarray(-sq[None, :], dtype=f32),
            "u": np.ascontiguousarray(u, dtype=f32),
            "v": np.ascontiguousarray(v_sb, dtype=f32),
            "cbase": cbase,
        })
    return in_maps


def kernel(x, conv_w, bn_gamma, bn_beta, bn_mean, bn_var):
    from concourse.bass_utils import run_bass_kernel_spmd

    x = np.asarray(x)
    in_maps = _host_prep(np.asarray(x, np.float32), np.asarray(conv_w),
                         np.asarray(bn_gamma), np.asarray(bn_beta),
                         np.asarray(bn_mean), np.asarray(bn_var))
    if "nc" not in _CACHED:
        _CACHED["nc"] = _build_bass()
    res = run_bass_kernel_spmd(_CACHED["nc"], in_maps, list(range(NCORES)))
    out = np.empty((B, O, N), np.float32)
    for core in range(NCORES):
        b, h = core // 2, core % 2
        out[b, :, h * HALF:(h + 1) * HALF] = res.results[core]["y"].T
    return out
